# revision 1
# baseline (speedup 1.0000x reference)
"""BitLinear (ternary-weight linear) kernel for Trainium2, 8 NeuronCores.

Computation:  out = x @ (w_ternary * scale)^T
  where scale = max(mean(|weight|), 1e-5)
        w_ternary = clip(round(weight / scale), -1, 1)  in {-1, 0, 1}

Strategy:
  - Host: quantize the 4 MB weight (tiny, elementwise) and pre-transpose it
    to wT [in, out]; scale is passed as a [1,1] tensor and applied by the
    scalar engine during the PSUM->SBUF output copy.
  - Device (data-parallel over the batch dim, 1 batch row per core):
    out_b = x_b @ wT with ternary +/-1 weights, fp32r matmuls (full PE rate
    at free dim >= 256, ~13 mantissa bits so +/-1 weights are exact and x
    carries ~6e-5 relative rounding).
    Per 128-row block of x_b: DMA the natural [128, 1024] tile, PE-transpose
    its 8 column slices (contraction dim must sit on partitions), DVE-copy
    the transposed slices out of PSUM, then 16 accumulating matmuls
    (lhsT = xT tile, rhs = wT slice) produce PSUM [128 s, 1024 o] which the
    scalar engine copies out with the scale applied, and DMA stores.
"""

import numpy as np

B, S, IN, OUT = 8, 8192, 1024, 1024
N_CORES = 8
P = 128
S_BLOCKS = S // P  # 64
K_TILES = IN // P  # 8
EPS = 1e-5

_compiled = None


def _build():
    import concourse.bacc as bacc
    import concourse.mybir as mybir
    import concourse.tile as tile

    R = mybir.dt.float32r
    F32 = mybir.dt.float32

    nc = bacc.Bacc()
    x = nc.declare_dram_parameter("x", [S, IN], R, isOutput=False)
    wt = nc.declare_dram_parameter("wt", [IN, OUT], R, isOutput=False)
    ident = nc.declare_dram_parameter("ident", [P, P], R, isOutput=False)
    scale_t = nc.declare_dram_parameter("scale", [1, 1], F32, isOutput=False)
    out = nc.declare_dram_parameter("out", [S, OUT], F32, isOutput=True)

    with tile.TileContext(nc) as tc:
        with (
            tc.tile_pool(name="const", bufs=1) as constp,
            tc.tile_pool(name="xn", bufs=3) as xnp,
            tc.tile_pool(name="xt", bufs=6) as xtp,
            tc.tile_pool(name="outp", bufs=3) as outp,
            tc.tile_pool(name="pst", bufs=4, space="PSUM") as pst,
            tc.tile_pool(name="pso", bufs=4, space="PSUM") as pso,
        ):
            ident_sb = constp.tile([P, P], R)
            nc.sync.dma_start(out=ident_sb, in_=ident[:])

            xn_tiles = {}

            def load_xn(b, halves=1):
                if b < S_BLOCKS and b not in xn_tiles:
                    t = xnp.tile([P, IN], R, tag="xn", name=f"xn_{b}")
                    hw = IN // halves
                    for i in range(halves):
                        nc.sync.dma_start(
                            out=t[:, i * hw:(i + 1) * hw],
                            in_=x[b * P:(b + 1) * P, i * hw:(i + 1) * hw],
                        )
                    xn_tiles[b] = t

            load_xn(0, halves=2)

            # Transposed ternary weight resident in SBUF: [128, k, 1024].
            # All startup DMAs go on the one Sync ring in priority order
            # (ident, x block 0, then weight k-slices interleaved with the
            # next x block) — a single ring drains strictly in order, so the
            # first transposes and first matmuls see their data earliest.
            wt_sb = constp.tile([P, K_TILES, OUT], R)
            wt_r = wt[:].rearrange("(a p) o -> p a o", p=P)
            for k in range(4):
                nc.sync.dma_start(
                    out=wt_sb[:, k:k + 1, :], in_=wt_r[:, k:k + 1, :]
                )
            load_xn(1)
            for k in range(4, K_TILES):
                nc.sync.dma_start(
                    out=wt_sb[:, k:k + 1, :], in_=wt_r[:, k:k + 1, :]
                )

            # scale broadcast to all 128 partitions for the scaled copy
            # (after the weight DMAs: the 128-way replicated write is slow
            # and must not delay the k=0 weight slice)
            scale_sb = constp.tile([P, 1], F32)
            nc.gpsimd.dma_start(
                out=scale_sb, in_=scale_t[:].to_broadcast((P, 1))
            )

            # Software-pipelined emission: the PE-transposes (+DVE copies)
            # for block b+1 are emitted BEFORE block b's matmuls, so the
            # copies complete during the 3.6us matmul phase and the next
            # block's first matmul never stalls on its transposed operand.
            def emit_transposes(b):
                # PE-transpose the 8 [128,128] column slices; pack 4 per
                # PSUM bank so 8 transposes only hold 2 banks.
                xn_sb = xn_tiles.pop(b)
                load_xn(b + 2)
                pts = [pst.tile([P, 4, P], R, tag="pst", name=f"pt{b}_{i}")
                       for i in range(2)]
                xts = [xtp.tile([P, 4, P], R, tag="xt4", name=f"xt{b}_{i}")
                       for i in range(2)]
                for i in range(2):
                    for j in range(4):
                        k = 4 * i + j
                        nc.tensor.transpose(
                            pts[i][:, j, :],
                            xn_sb[:, k * P:(k + 1) * P],
                            ident_sb,
                        )
                    nc.vector.tensor_copy(xts[i], pts[i])
                return xts

            xts_cur = emit_transposes(0)
            for b in range(S_BLOCKS):
                xts_next = (emit_transposes(b + 1)
                            if b + 1 < S_BLOCKS else None)

                # h-outer: finish the o-half-0 accumulation first so its
                # scaled copy + store overlap the o-half-1 matmuls; per-
                # element k order is unchanged, so numerics are identical.
                out_sb = outp.tile([P, OUT], F32)
                for h in range(2):
                    po_h = pso.tile([P, 512], F32, tag="pso",
                                    name=f"po{b}_{h}")
                    for k in range(K_TILES):
                        nc.tensor.matmul(
                            po_h,
                            lhsT=xts_cur[k // 4][:, k % 4, :],
                            rhs=wt_sb[:, k, h * 512:(h + 1) * 512],
                            start=(k == 0),
                            stop=(k == K_TILES - 1),
                        )
                    # last block's final half drains in 256-wide chunks
                    # so the closing copy->store chain is shorter
                    n_chunks = 2 if (b == S_BLOCKS - 1 and h == 1) else 1
                    cw = 512 // n_chunks
                    for c in range(n_chunks):
                        lo = h * 512 + c * cw
                        nc.scalar.activation(
                            out_sb[:, lo:lo + cw],
                            po_h[:, c * cw:(c + 1) * cw],
                            mybir.ActivationFunctionType.Copy,
                            scale=scale_sb[:, 0:1],
                        )
                        nc.sync.dma_start(
                            out=out[b * P:(b + 1) * P, lo:lo + cw],
                            in_=out_sb[:, lo:lo + cw],
                        )
                xts_cur = xts_next
    nc.finalize()
    return nc


def _get_compiled():
    global _compiled
    if _compiled is None:
        _compiled = _build()
    return _compiled


def quantize_host(weight: np.ndarray):
    """Mirror of the reference ste_quantize, done on host in fp32.

    The mean is computed in float64 then rounded to fp32 so it tracks the
    true mean more closely than any fp32 summation order.
    """
    scale = np.float32(max(np.mean(np.abs(weight), dtype=np.float64), EPS))
    w_t = np.clip(np.round(weight / scale), -1.0, 1.0).astype(np.float32)
    return w_t, scale


def kernel(x: np.ndarray, weight: np.ndarray) -> np.ndarray:
    from concourse.bass_utils import run_bass_kernel_spmd

    x = np.asarray(x, dtype=np.float32)
    weight = np.asarray(weight, dtype=np.float32)
    assert x.shape == (B, S, IN) and weight.shape == (OUT, IN)
    w_t, scale = quantize_host(weight)
    wt_T = np.ascontiguousarray(w_t.T)  # [in, out]
    ident = np.eye(P, dtype=np.float32)
    scale_arr = np.array([[scale]], dtype=np.float32)

    nc = _get_compiled()
    in_maps = [
        {"x": np.ascontiguousarray(x[c]), "wt": wt_T, "ident": ident,
         "scale": scale_arr}
        for c in range(N_CORES)
    ]
    res = run_bass_kernel_spmd(nc, in_maps, core_ids=list(range(N_CORES)))
    return np.stack([res.results[c]["out"] for c in range(N_CORES)], axis=0)



# revision 2
# speedup vs baseline: 1.4937x; 1.4937x over previous
"""BitLinear (ternary-weight linear) kernel for Trainium2, 8 NeuronCores.

Computation:  out = x @ (w_ternary * scale)^T
  where scale = max(mean(|weight|), 1e-5)
        w_ternary = clip(round(weight / scale), -1, 1)  in {-1, 0, 1}

Strategy (v2):
  - Host: quantize the 4 MB weight, pre-transpose it to wT [in, out] in
    bf16 (ternary values are exact in bf16), and pre-transpose each
    core's x slice to xT [in, s] in bf16.  bf16 x rounding gives
    ~1.5e-3 max-rel output error (tolerance 2e-2); the fp32 scale is
    applied by the scalar engine during the PSUM->SBUF output copy.
  - Device (data-parallel over batch, 1 batch row per core): pure GEMM,
    no on-chip transposes.  Per 512-column s-chunk: DMA the 8 k-tiles
    of xT (1 KB/partition lines), then for each of the 4 s-blocks run
    2 PSUM halves x 8 accumulating bf16 matmuls (lhsT = xT slice
    [128 i, 128 s] stationary, rhs = wT slice [128 i, 512 o] moving),
    scalar-copy with scale, DMA store.  bf16 matmuls issue at ~216 ns
    (FWL hides LDWEIGHTS); PE is the bottleneck at ~221 us.
"""

import numpy as np

B, S, IN, OUT = 8, 8192, 1024, 1024
N_CORES = 8
P = 128
SC = 512                 # s-chunk width
N_CHUNKS = S // SC       # 16
BLOCKS_PER_CHUNK = SC // P  # 4
K_TILES = IN // P        # 8
EPS = 1e-5

_compiled = None


def _build():
    import concourse.bacc as bacc
    import concourse.mybir as mybir
    import concourse.tile as tile

    BF = mybir.dt.bfloat16
    F32 = mybir.dt.float32

    nc = bacc.Bacc()
    xt = nc.declare_dram_parameter("xt", [IN, S], BF, isOutput=False)
    wt = nc.declare_dram_parameter("wt", [IN, OUT], BF, isOutput=False)
    scale_t = nc.declare_dram_parameter("scale", [1, 1], F32, isOutput=False)
    out = nc.declare_dram_parameter("out", [S, OUT], F32, isOutput=True)

    with tile.TileContext(nc) as tc:
        with (
            tc.tile_pool(name="const", bufs=1) as constp,
            tc.tile_pool(name="xtp", bufs=3) as xtp,
            tc.tile_pool(name="outp", bufs=4) as outp,
            tc.tile_pool(name="pso", bufs=4, space="PSUM") as pso,
        ):
            xt_r = xt[:].rearrange("(a p) s -> p a s", p=P)
            wt_r = wt[:].rearrange("(a p) o -> p a o", p=P)

            xt_tiles = {}

            def load_chunk(c):
                if c < N_CHUNKS and c not in xt_tiles:
                    t = xtp.tile([P, K_TILES, SC], BF, tag="xt",
                                 name=f"xt_{c}")
                    for k in range(K_TILES):
                        nc.sync.dma_start(
                            out=t[:, k, :],
                            in_=xt_r[:, k, c * SC:(c + 1) * SC],
                        )
                    xt_tiles[c] = t

            # startup order on the single sync ring: first chunk's x
            # k-tiles interleaved with the weight k-slices so the k=0
            # matmuls see their operands earliest.
            wt_sb = constp.tile([P, K_TILES, OUT], BF)
            load_chunk(0)
            for k in range(K_TILES):
                nc.sync.dma_start(out=wt_sb[:, k, :], in_=wt_r[:, k, :])
            load_chunk(1)

            scale_sb = constp.tile([P, 1], F32)
            nc.gpsimd.dma_start(
                out=scale_sb, in_=scale_t[:].to_broadcast((P, 1))
            )

            for c in range(N_CHUNKS):
                xt_sb = xt_tiles.pop(c)
                load_chunk(c + 2)
                for sb in range(BLOCKS_PER_CHUNK):
                    lo_s = sb * P
                    out_sb = outp.tile([P, OUT], F32)
                    s0 = c * SC + sb * P
                    for h in range(2):
                        po_h = pso.tile([P, 512], F32, tag="pso",
                                        name=f"po{c}_{sb}_{h}")
                        for k in range(K_TILES):
                            nc.tensor.matmul(
                                po_h,
                                lhsT=xt_sb[:, k, lo_s:lo_s + P],
                                rhs=wt_sb[:, k, h * 512:(h + 1) * 512],
                                start=(k == 0),
                                stop=(k == K_TILES - 1),
                            )
                        # last block's final half drains in 256-wide
                        # chunks so the closing copy->store chain is short
                        last = (c == N_CHUNKS - 1
                                and sb == BLOCKS_PER_CHUNK - 1)
                        n_chunks = 2 if (last and h == 1) else 1
                        cw = 512 // n_chunks
                        for cc in range(n_chunks):
                            lo = h * 512 + cc * cw
                            nc.scalar.activation(
                                out_sb[:, lo:lo + cw],
                                po_h[:, cc * cw:(cc + 1) * cw],
                                mybir.ActivationFunctionType.Copy,
                                scale=scale_sb[:, 0:1],
                            )
                            nc.sync.dma_start(
                                out=out[s0:s0 + P, lo:lo + cw],
                                in_=out_sb[:, lo:lo + cw],
                            )
    nc.finalize()
    return nc


def _get_compiled():
    global _compiled
    if _compiled is None:
        _compiled = _build()
    return _compiled


def quantize_host(weight: np.ndarray):
    """Mirror of the reference ste_quantize, done on host in fp32.

    The mean is computed in float64 then rounded to fp32 so it tracks the
    true mean more closely than any fp32 summation order.
    """
    scale = np.float32(max(np.mean(np.abs(weight), dtype=np.float64), EPS))
    w_t = np.clip(np.round(weight / scale), -1.0, 1.0).astype(np.float32)
    return w_t, scale


def make_in_maps(x: np.ndarray, weight: np.ndarray):
    import ml_dtypes

    w_t, scale = quantize_host(weight)
    wt_T = np.ascontiguousarray(w_t.T).astype(ml_dtypes.bfloat16)  # [in, out]
    scale_arr = np.array([[scale]], dtype=np.float32)
    # per-core transposed bf16 activations [in, s]
    xbt = np.ascontiguousarray(x.transpose(0, 2, 1)).astype(ml_dtypes.bfloat16)
    return [
        {"xt": xbt[c], "wt": wt_T, "scale": scale_arr}
        for c in range(N_CORES)
    ], scale


def kernel(x: np.ndarray, weight: np.ndarray) -> np.ndarray:
    from concourse.bass_utils import run_bass_kernel_spmd

    x = np.asarray(x, dtype=np.float32)
    weight = np.asarray(weight, dtype=np.float32)
    assert x.shape == (B, S, IN) and weight.shape == (OUT, IN)
    in_maps, _ = make_in_maps(x, weight)
    nc = _get_compiled()
    res = run_bass_kernel_spmd(nc, in_maps, core_ids=list(range(N_CORES)))
    return np.stack([res.results[c]["out"] for c in range(N_CORES)], axis=0)
